# revision 48
# baseline (speedup 1.0000x reference)
"""Bidirectional Mamba block on 8 Trainium2 NeuronCores (Bass/Tile).

Data-parallel over batch: B=16 -> 2 per core; weights replicated; host gathers.
Per-core layout is feature-major ([feature_partitions, tokens]) with tokens =
batch-major concatenation of the 2 local sequences (t = b*512 + l).

Engine assignment (v2):
  PE   - all projections (weights stationary as lhsT), depthwise causal conv as
         4 accumulating diag-matmuls over shifted views, y = sum_n h*C via
         identity-matmul PSUM accumulation for all 4 d-tiles (8 banks).
  ACT  - silu via the resident Silu table (1 op), softplus = ln(exp(.)+1),
         dA_n = exp(delta * A[:,n]) with per-partition scale, PSUM->SBUF
         copies; rsqrt = exp(-0.5*ln(.)).
  DVE  - selective scan via tensor_tensor_scan (fp32 internal state), one
         merged scan per (n,dt) covering both local sequences via a
         boundary-poisoned delta column; backward layer feeds globally
         reversed access patterns; all bx/p muls (Pool measured slower).
  DMA  - B/C per-token row broadcasts to 128 partitions via stride-0-free-dim
         SBUF->SBUF dma, two-stage to spread SBUF port load (replaces PE
         one-hot matmul + ACT copy).
The body is emitted so iteration i+1 of the timing loop overlaps iteration
i's tail: xT double-buffered, head PSUM peak 4 banks, front-phase pools
parked on banks 4-7.
"""

import numpy as np

# ---- problem constants (hardcoded per contract) ----
B, L, DM = 16, 512, 256
DI, N, R, KC = 512, 16, 16, 4
NCORES = 8
BL = B // NCORES          # local batch
TOK = BL * L              # 1024 tokens per core
DT_TILES = DI // 128      # 4
MT = DM // 128            # 2
F32_np = np.float32

import ml_dtypes
BF16_np = ml_dtypes.bfloat16

CFG = dict(
    # Engine split for the scan-loop muls. Measured HW rates per [128,1024]:
    # DVE mul 657ns, Pool mul 2139ns, DVE scan-pair 2301ns, ACT exp 1273ns.
    # Any Pool usage in the scan loop measured SLOWER end-to-end than pure
    # DVE (cross-engine stalls), so Pool stays off.
    POOL_BX=lambda n, dt: False,
    POOL_P=lambda n, dt: False,
    POOL_YADD=lambda n: False,
    N_PE_DT=3,      # d-tiles accumulated on PE/PSUM (2 banks each)
    DA_BUFS=8,      # dA pool depth (ACT run-ahead)
    SCAN_BUFS=3,
    REP_BUFS=3,
    NCHUNK1=12,      # insert layer-b silu phases after this many f-scan n's
    PROBE="",
)

_BUILD_CACHE = {}


# ======================================================================
# host-side weight preparation
# ======================================================================

def _prep_layer_weights(inw, convw, convb, xprojw, dtw, dtb, Alog, Dp, outw, normw):
    """Fold/reshape one mamba layer's weights into device layouts."""
    out = {}
    # in_proj with rmsnorm weight folded into rows: [128, 2, 1024]
    w = (np.asarray(normw)[:, None] * np.asarray(inw)).astype(F32_np)
    out["inw"] = np.ascontiguousarray(w.reshape(2, 128, 2 * DI).transpose(1, 0, 2)).astype(BF16_np)
    # conv diag matrices: [128, 16(dt*4+k), 128]
    cd = np.zeros((128, DT_TILES * KC, 128), F32_np)
    cw = np.asarray(convw).astype(F32_np)  # (KC, 1, DI)
    for dt in range(DT_TILES):
        for k in range(KC):
            idx = np.arange(128)
            cd[idx, dt * KC + k, idx] = cw[k, 0, dt * 128 + idx]
    out["convd"] = np.ascontiguousarray(cd).astype(BF16_np)
    out["convb"] = np.ascontiguousarray(
        np.asarray(convb).astype(F32_np).reshape(DT_TILES, 128, 1).transpose(1, 0, 2))
    # xproj padded so delta_raw/B/C land at partitions 0/32/64: [128, 4, 96]
    xp = np.zeros((DI, 96), F32_np)
    xpw = np.asarray(xprojw).astype(F32_np)
    xp[:, 0:R] = xpw[:, 0:R]
    xp[:, 32:32 + N] = xpw[:, R:R + N]
    xp[:, 64:64 + N] = xpw[:, R + N:R + 2 * N]
    out["xpw"] = np.ascontiguousarray(xp.reshape(DT_TILES, 128, 96).transpose(1, 0, 2)).astype(BF16_np)
    out["dtw"] = np.ascontiguousarray(np.asarray(dtw).astype(F32_np)).astype(BF16_np)          # (16, 512)
    out["dtb"] = np.ascontiguousarray(
        np.asarray(dtb).astype(F32_np).reshape(DT_TILES, 128, 1).transpose(1, 0, 2))
    A = (-np.exp(np.asarray(Alog).astype(np.float64))).astype(F32_np)          # (512, 16)
    out["A"] = np.ascontiguousarray(A.reshape(DT_TILES, 128, N).transpose(1, 0, 2))
    out["Dp"] = np.ascontiguousarray(
        np.asarray(Dp).astype(F32_np).reshape(DT_TILES, 128, 1).transpose(1, 0, 2))
    out["outw"] = np.ascontiguousarray(
        np.asarray(outw).astype(F32_np).reshape(DT_TILES, 128, DM).transpose(1, 0, 2)).astype(BF16_np)
    return out


def _prep_shared_weights(proj_w, proj_b, ln_g, ln_b):
    out = {}
    out["projw"] = np.ascontiguousarray(
        np.asarray(proj_w).astype(F32_np).reshape(4, 128, DM).transpose(1, 0, 2)).astype(BF16_np)
    out["projb"] = np.ascontiguousarray(
        np.asarray(proj_b).astype(F32_np).reshape(MT, 128, 1).transpose(1, 0, 2))
    out["lng"] = np.ascontiguousarray(
        np.asarray(ln_g).astype(F32_np).reshape(MT, 128, 1).transpose(1, 0, 2))
    out["lnb"] = np.ascontiguousarray(
        np.asarray(ln_b).astype(F32_np).reshape(MT, 128, 1).transpose(1, 0, 2))
    return out


# ======================================================================
# device program
# ======================================================================

def _build(loop_k=1, cfg=None):
    cfg = dict(CFG if cfg is None else cfg)
    key = (loop_k, tuple(sorted((k, str(v)) for k, v in cfg.items())))
    if key in _BUILD_CACHE:
        return _BUILD_CACHE[key]

    import concourse.bacc as bacc
    import concourse.mybir as mybir
    import concourse.tile as tile
    import concourse.bass as cbass

    F32 = mybir.dt.float32
    BF16 = mybir.dt.bfloat16
    AF = mybir.ActivationFunctionType
    ALU = mybir.AluOpType

    nc = bacc.Bacc("TRN2", target_bir_lowering=False, debug=False)

    def din(name, shape, dt=None):
        return nc.dram_tensor(name, list(shape), dt or F32, kind="ExternalInput").ap()

    # --- DRAM I/O ---
    xT_d = din("xT", (DM, TOK))
    lw_d = {}
    for s in ("f", "b"):
        lw_d[s] = {
            "inw": din(f"{s}_inw", (128, 2, 2 * DI), BF16),
            "convd": din(f"{s}_convd", (128, DT_TILES * KC, 128), BF16),
            "convb": din(f"{s}_convb", (128, DT_TILES, 1)),
            "xpw": din(f"{s}_xpw", (128, DT_TILES, 96), BF16),
            "dtw": din(f"{s}_dtw", (16, DI), BF16),
            "dtb": din(f"{s}_dtb", (128, DT_TILES, 1)),
            "A": din(f"{s}_A", (128, DT_TILES, N)),
            "Dp": din(f"{s}_Dp", (128, DT_TILES, 1)),
            "outw": din(f"{s}_outw", (128, DT_TILES, DM), BF16),
        }
    projw_d = din("projw", (128, 4, DM), BF16)
    projb_d = din("projb", (128, MT, 1))
    lng_d = din("lng", (128, MT, 1))
    lnb_d = din("lnb", (128, MT, 1))
    outT_d = nc.dram_tensor("outT", [DM, TOK], F32, kind="ExternalOutput").ap()

    PAD = KC - 1  # 3
    CONVW = 2 * PAD + L  # padded per-batch row length 518

    def rep_ap(src, nrep):
        """AP repeating each of src's partitions nrep times via a stride-0
        free dim (row-identical content makes dst partition order moot)."""
        return cbass.AP(
            tensor=src.tensor, offset=src.offset,
            ap=[list(src.ap[0]), [0, nrep]] + [list(x) for x in src.ap[1:]])

    with tile.TileContext(nc) as tc:
        from contextlib import ExitStack
        with ExitStack() as ctx:
            wpool = ctx.enter_context(tc.tile_pool(name="wpool", bufs=1))
            pers = ctx.enter_context(tc.tile_pool(name="pers", bufs=1))
            # double-buffered so iteration i+1's xT load + front phases can
            # start while iteration i's tail still reads/writes its xT
            xtp = ctx.enter_context(tc.tile_pool(name="xtp", bufs=2))
            work = ctx.enter_context(tc.tile_pool(name="work", bufs=2))
            rep = ctx.enter_context(tc.tile_pool(name="rep", bufs=cfg["REP_BUFS"]))
            dapool = ctx.enter_context(tc.tile_pool(name="dapool", bufs=cfg["DA_BUFS"]))
            scanw = ctx.enter_context(tc.tile_pool(name="scanw", bufs=cfg["SCAN_BUFS"]))

            def body():
                # ---- load shared weights ----
                projw_t = wpool.tile([128, 4, DM], BF16, tag="projw", name="projw")
                nc.sync.dma_start(projw_t[:], projw_d[:])
                projb_t = wpool.tile([128, MT, 1], F32, tag="projb", name="projb")
                nc.sync.dma_start(projb_t[:], projb_d[:])
                lng_t = wpool.tile([128, MT, 1], F32, tag="lng", name="lng")
                nc.sync.dma_start(lng_t[:], lng_d[:])
                lnb_t = wpool.tile([128, MT, 1], F32, tag="lnb", name="lnb")
                nc.sync.dma_start(lnb_t[:], lnb_d[:])
                idn = wpool.tile([128, 128], BF16, tag="idn", name="idn")
                from concourse.masks import make_identity
                make_identity(nc, idn[:])
                idnf = wpool.tile([128, 128], F32, tag="idnf", name="idnf")
                make_identity(nc, idnf[:])

                # per-layer weights (both layers up front; distinct tags)
                lw = {}
                for s in ("f", "b"):
                    W = lw_d[s]
                    t = {}
                    t["inw"] = wpool.tile([128, 2, 2 * DI], BF16, tag=f"{s}inw", name=f"{s}inw")
                    t["convd"] = wpool.tile([128, DT_TILES * KC, 128], BF16, tag=f"{s}convd", name=f"{s}convd")
                    t["convb"] = wpool.tile([128, DT_TILES, 1], F32, tag=f"{s}convb", name=f"{s}convb")
                    t["xpw"] = wpool.tile([128, DT_TILES, 96], BF16, tag=f"{s}xpw", name=f"{s}xpw")
                    t["dtw"] = wpool.tile([16, DI], BF16, tag=f"{s}dtw", name=f"{s}dtw")
                    t["dtb"] = wpool.tile([128, DT_TILES, 1], F32, tag=f"{s}dtb", name=f"{s}dtb")
                    t["A"] = wpool.tile([128, DT_TILES, N], F32, tag=f"{s}A", name=f"{s}A")
                    t["Dp"] = wpool.tile([128, DT_TILES, 1], F32, tag=f"{s}Dp", name=f"{s}Dp")
                    t["outw"] = wpool.tile([128, DT_TILES, DM], BF16, tag=f"{s}outw", name=f"{s}outw")
                    for k2, tl in t.items():
                        nc.sync.dma_start(tl[:], W[k2][:])
                    lw[s] = t

                xT = []
                for m in range(MT):
                    t = xtp.tile([128, TOK], F32, tag=f"xT{m}", name=f"xT{m}")
                    nc.sync.dma_start(t[:], xT_d[m * 128:(m + 1) * 128, :])
                    xT.append(t)
                # parks the low 4 PSUM banks so the front-phase pools land on
                # banks 4-7, disjoint from the previous iteration's tail pools
                dummy_ctx = tc.tile_pool(name="park", bufs=1, space="PSUM")
                park = dummy_ctx.__enter__()
                park_t = park.tile([1, 2048], F32, tag="park", name="park")
                nc.vector.memset(park_t[:, 0:1], 0.0)

                # ---- shared RMSNorm: xn = x * rsqrt(mean(x^2) + eps) ----
                xn = []
                with tc.tile_pool(name="prms", bufs=1, space="PSUM") as prms:
                    ones_col = wpool.tile([128, 1], F32, tag="ones_col", name="ones_col")
                    nc.vector.memset(ones_col[:], 1.0)
                    ss_ps = prms.tile([1, TOK], F32, tag="ss", name="ss")
                    for fh in range(2):
                        fs = slice(fh * 512, (fh + 1) * 512)
                        for m in range(MT):
                            sq = work.tile([128, 512], F32, tag="sqtmp", name="rms_sq")
                            nc.scalar.square(sq[:], xT[m][:, fs])
                            nc.tensor.matmul(ss_ps[:, fs], ones_col[:], sq[:],
                                             start=(m == 0), stop=(m == MT - 1))
                    # rs = exp(-0.5 * ln(ss/DM + eps))
                    eps1 = wpool.tile([1, 1], F32, tag="eps1", name="eps1")
                    nc.vector.memset(eps1[:], 1e-5)
                    rs_row = work.tile([1, TOK], F32, tag="rowtmp", name="rs_row")
                    nc.scalar.activation(rs_row[:], ss_ps[:], AF.Ln,
                                         scale=1.0 / DM, bias=eps1[:, 0:1])
                    nc.scalar.activation(rs_row[:], rs_row[:], AF.Exp, scale=-0.5)
                    ones1 = wpool.tile([1, 128], F32, tag="ones1", name="ones1")
                    nc.vector.memset(ones1[:], 1.0)
                    rs_ps = prms.tile([128, TOK], F32, tag="rs_rep", name="rs_rep")
                    for fh in range(2):
                        fs = slice(fh * 512, (fh + 1) * 512)
                        nc.tensor.matmul(rs_ps[:, fs], ones1[:], rs_row[:, fs],
                                         start=True, stop=True)
                    for m in range(MT):
                        t = pers.tile([128, TOK], BF16, tag=f"xn{m}", name=f"xn{m}")
                        nc.vector.tensor_mul(t[:], xT[m][:], rs_ps[:])
                        xn.append(t)

                # per-layer persistent state
                st = {s: {} for s in ("f", "b")}

                # ---- phase 1+2: in_proj + conv (silu table) ----
                def inproj_conv(s, reverse):
                    W = lw[s]
                    S = st[s]
                    xmpad = []
                    S["sz"] = sz = []
                    S["xs"] = xs = []
                    for dt in range(DT_TILES):
                        # shared tag across layers: serialized by emission order
                        t = pers.tile([128, BL, CONVW], BF16, tag=f"xmpad{dt}", name=f"xmpad{s}{dt}")
                        if s == "f":
                            nc.vector.memset(t[:, :, 0:PAD], 0.0)
                            nc.vector.memset(t[:, :, PAD + L:CONVW], 0.0)
                        xmpad.append(t)
                        sz.append(pers.tile([128, TOK], BF16, tag=f"{s}sz{dt}", name=f"{s}sz{dt}"))
                        xs.append(pers.tile([128, TOK], BF16, tag=f"{s}xs{dt}", name=f"{s}xs{dt}"))

                    with tc.tile_pool(name=f"pp{s}", bufs=4 if s == "f" else 2,
                                      space="PSUM") as pp:
                        for m in range(8):
                            for fh in range(2):
                                fs = slice(fh * 512, (fh + 1) * 512)
                                ps = pp.tile([128, 512], F32, tag="pp", name="pp")
                                for ks in range(2):
                                    nc.tensor.matmul(
                                        ps[:], W["inw"][:, ks, m * 128:(m + 1) * 128],
                                        xn[ks][:, fs], start=(ks == 0), stop=(ks == 1))
                                if m < 4:
                                    nc.scalar.copy(xmpad[m][:, fh, PAD:PAD + L], ps[:])
                                else:
                                    nc.scalar.activation(sz[m - 4][:, fs], ps[:], AF.Silu)

                        # depthwise causal conv + silu
                        for dt in range(DT_TILES):
                            for b in range(BL):
                                ps = pp.tile([128, 512], F32, tag="pp", name="pp")
                                for k in range(KC):
                                    off = k if not reverse else (2 * PAD - k)
                                    nc.tensor.matmul(
                                        ps[:], W["convd"][:, dt * KC + k, :],
                                        xmpad[dt][:, b, off:off + L],
                                        start=(k == 0), stop=(k == KC - 1))
                                bs = slice(b * L, (b + 1) * L)
                                nc.scalar.activation(xs[dt][:, bs], ps[:], AF.Silu,
                                                     bias=W["convb"][:, dt, 0:1])

                # ---- phase 3+4: xproj -> B/C rows (+dma broadcasts), dt_proj ----
                def xproj_dt(s, reverse=False):
                    W = lw[s]
                    S = st[s]
                    # tag shared across layers: b's writes land after f's scan
                    # has consumed dbc_f (emission order enforces the WAR dep)
                    dbc = pers.tile([16, 2, TOK], BF16, tag="dbc", name=f"{s}dbc")
                    S["dbc"] = dbc
                    draw_t = work.tile([16, TOK], BF16, tag="draw", name=f"{s}draw")
                    draw = draw_t[:, :]
                    xs = S["xs"]
                    with tc.tile_pool(name=f"pxp{s}", bufs=1, space="PSUM") as pxp:
                        psx = pxp.tile([96, TOK], F32, tag="pxp", name="pxp")
                        for fh in range(2):
                            fs = slice(fh * 512, (fh + 1) * 512)
                            for ks in range(DT_TILES):
                                nc.tensor.matmul(psx[:, fs], W["xpw"][:, ks, :],
                                                 xs[ks][:, fs],
                                                 start=(ks == 0), stop=(ks == DT_TILES - 1))
                        nc.scalar.copy(draw, psx[0:16, :])
                        nc.scalar.copy(dbc[:, 0, :], psx[32:48, :])
                        nc.scalar.copy(dbc[:, 1, :], psx[64:80, :])
                    # broadcast stage 1: row n -> partitions 8n..8n+7
                    dbc8 = pers.tile([128, 2, TOK], BF16, tag="dbc8", name=f"{s}dbc8")
                    S["dbc8"] = dbc8
                    nc.sync.dma_start(dbc8[:], rep_ap(dbc[:], 8))

                    # dt_proj + softplus -> delta; w = delta * xs
                    S["delta"] = delta = []
                    S["w"] = w_t = []
                    with tc.tile_pool(name=f"pdt{s}", bufs=3 if s == "f" else 2,
                                      space="PSUM") as pdt:
                        for dt in range(DT_TILES):
                            dl = pers.tile([128, TOK], BF16, tag=f"delta{dt}", name=f"{s}delta{dt}")
                            for fh in range(2):
                                fs = slice(fh * 512, (fh + 1) * 512)
                                ps = pdt.tile([128, 512], F32, tag="pdt", name="pdt")
                                nc.tensor.matmul(ps[:], W["dtw"][:, dt * 128:(dt + 1) * 128],
                                                 draw[:, fs], start=True, stop=True)
                                e = work.tile([128, 512], F32, tag="detag", name="de")
                                nc.scalar.activation(e[:], ps[:], AF.Exp,
                                                     bias=W["dtb"][:, dt, 0:1])
                                nc.scalar.activation(dl[:, fs], e[:], AF.Ln, bias=1.0)
                            delta.append(dl)
                            wt = pers.tile([128, TOK], BF16, tag=f"w{dt}", name=f"{s}w{dt}")
                            nc.vector.tensor_mul(wt[:], dl[:], xs[dt][:])
                            w_t.append(wt)
                            # poison the boundary step AFTER w is computed:
                            # dA = exp(88*A_n) ~= 0 there for every n, so one
                            # merged scan covers both local sequences
                            bcol = L if not reverse else (L - 1)
                            nc.vector.memset(dl[:, bcol:bcol + 1], 88.0)

                # ---- phase 5: selective scan (chunked over n for interleaving) ----
                NPE = cfg["N_PE_DT"]

                def scan_open(s):
                    pool = tc.tile_pool(name=f"pyac{s}", bufs=1, space="PSUM")
                    pyac = pool.__enter__()
                    y_ps = [pyac.tile([128, TOK], F32, tag=f"yps{dt}", name=f"{s}yps{dt}")
                            for dt in range(NPE)]
                    st[s]["y_ps"] = y_ps
                    for dt in range(NPE, DT_TILES):
                        st[s][f"y_acc{dt}"] = pers.tile(
                            [128, TOK], BF16, tag=f"yacc{dt}", name=f"{s}yacc{dt}")
                    return pool

                def scan_chunk(s, reverse, n0, n1):
                    S = st[s]
                    W = lw[s]
                    delta, w_t, dbc8, y_ps = S["delta"], S["w"], S["dbc8"], S["y_ps"]
                    pool_bx, pool_p = cfg["POOL_BX"], cfg["POOL_P"]
                    for n in range(n0, n1):
                        bc = rep.tile([128, 2, TOK], BF16, tag="bc", name="bc")
                        nc.sync.dma_start(bc[:], rep_ap(dbc8[8 * n:8 * n + 8, :, :], 16))
                        B_rep = bc[:, 0, :]
                        C_rep = bc[:, 1, :]
                        for dt in range(DT_TILES):
                            dA = dapool.tile([128, TOK], BF16, tag="dA", name="dA")
                            nc.scalar.activation(dA[:], delta[dt][:], AF.Exp,
                                                 scale=W["A"][:, dt, n:n + 1])
                            bx = scanw.tile([128, TOK], BF16, tag="bx", name="bx")
                            eng = nc.gpsimd if pool_bx(n, dt) else nc.vector
                            eng.tensor_mul(bx[:], w_t[dt][:], B_rep[:])
                            # merged scan; for reverse the stream is globally
                            # time-flipped so h[k] = h(t=TOK-1-k)
                            h = scanw.tile([128, TOK], BF16, tag="h", name="h")
                            if not reverse:
                                nc.vector.tensor_tensor_scan(
                                    h[:], dA[:], bx[:], 0.0, ALU.mult, ALU.add)
                            else:
                                nc.vector.tensor_tensor_scan(
                                    h[:], dA[:, ::-1], bx[:, ::-1], 0.0,
                                    ALU.mult, ALU.add)
                            eng = nc.gpsimd if pool_p(n, dt) else nc.vector
                            if dt >= NPE and n == 0:
                                p = S[f"y_acc{dt}"]
                            else:
                                p = scanw.tile([128, TOK], BF16, tag="p", name="p")
                            if not reverse:
                                eng.tensor_mul(p[:], h[:], C_rep[:])
                            else:
                                eng.tensor_mul(p[:], h[:, ::-1], C_rep[:])
                            if dt < NPE:
                                # stop stays False: gate() appends the Dp*xs
                                # skip-term matmul which closes the group
                                for fh in range(2):
                                    fs = slice(fh * 512, (fh + 1) * 512)
                                    nc.tensor.matmul(y_ps[dt][:, fs], idn[:], p[:, fs],
                                                     start=(n == 0), stop=False)
                            elif n > 0:
                                ya = S[f"y_acc{dt}"]
                                aeng = nc.gpsimd if cfg["POOL_YADD"](n) else nc.vector
                                aeng.tensor_add(ya[:], ya[:], p[:])

                # ---- phase 6a: gate (y + Dp*xs, * silu(z)) in place on xs ----
                def gate(s):
                    S = st[s]
                    W = lw[s]
                    for dt in range(DT_TILES):
                        if dt < NPE:
                            # y_ps += Dp*xs on PE (ACT pre-scales, recycling
                            # the p-tag buffers), so the gate is one mul
                            tmp = scanw.tile([128, TOK], BF16, tag="p", name="dpxs")
                            nc.scalar.activation(tmp[:], S["xs"][dt][:], AF.Identity,
                                                 scale=W["Dp"][:, dt, 0:1])
                            for fh in range(2):
                                fs = slice(fh * 512, (fh + 1) * 512)
                                nc.tensor.matmul(S["y_ps"][dt][:, fs], idn[:],
                                                 tmp[:, fs], start=False, stop=True)
                            nc.vector.tensor_mul(S["xs"][dt][:], S["y_ps"][dt][:],
                                                 S["sz"][dt][:])
                        else:
                            nc.vector.scalar_tensor_tensor(
                                S["xs"][dt][:], S["xs"][dt][:], W["Dp"][:, dt, 0:1],
                                S[f"y_acc{dt}"][:], ALU.mult, ALU.add)
                            nc.vector.tensor_mul(S["xs"][dt][:], S["xs"][dt][:],
                                                 S["sz"][dt][:])

                # ---- phase 6b: out_proj + residual ----
                def out_proj(s, bufs=3):
                    S = st[s]
                    W = lw[s]
                    g = S["xs"]
                    xout = []
                    with tc.tile_pool(name=f"po{s}", bufs=bufs, space="PSUM") as po:
                        for m in range(MT):
                            t = pers.tile([128, TOK], BF16, tag=f"x{s}out{m}", name=f"x{s}out{m}")
                            for fh in range(2):
                                fs = slice(fh * 512, (fh + 1) * 512)
                                ps = po.tile([128, 512], F32, tag="po", name="po")
                                for ks in range(DT_TILES):
                                    nc.tensor.matmul(
                                        ps[:], W["outw"][:, ks, m * 128:(m + 1) * 128],
                                        g[ks][:, fs], start=(ks == 0), stop=False)
                                # residual folded into the PSUM group (f32)
                                nc.tensor.matmul(ps[:], idnf[:], xT[m][:, fs],
                                                 start=False, stop=True)
                                nc.scalar.copy(t[:, fs], ps[:])
                            xout.append(t)
                    return xout

                # ======== emission schedule ========
                inproj_conv("f", reverse=False)        # silu table
                xproj_dt("f", reverse=False)           # exp/ln table
                dummy_ctx.__exit__(None, None, None)   # release parked banks
                pyf = scan_open("f")
                nck = cfg["NCHUNK1"]
                scan_chunk("f", False, 0, nck)         # exp
                inproj_conv("b", reverse=True)         # silu (ACT switches)
                scan_chunk("f", False, nck, N)         # exp
                xproj_dt("b", reverse=True)            # exp/ln
                gate("f")
                pyf.__exit__(None, None, None)
                pyb = scan_open("b")          # 6 banks; pof below fits in 2
                x1 = out_proj("f", bufs=2)    # overlaps scan_b
                scan_chunk("b", True, 0, N)
                gate("b")
                pyb.__exit__(None, None, None)
                x2 = out_proj("b")

                if cfg["PROBE"] == "nohead":
                    for m in range(MT):
                        nc.gpsimd.dma_start(outT_d[m * 128:(m + 1) * 128, :], x1[m][:])
                    return
                # ---- head: relu(cat(x1,x2) @ proj_w + proj_b), residual, layernorm ----
                cat = x1 + x2
                xn2 = xT     # residual accumulates in place onto xT
                with tc.tile_pool(name="ph", bufs=3, space="PSUM") as ph:
                    for m in range(MT):
                        for fh in range(2):
                            fs = slice(fh * 512, (fh + 1) * 512)
                            ps = ph.tile([128, 512], F32, tag="ph", name="ph")
                            for ks in range(4):
                                nc.tensor.matmul(
                                    ps[:], projw_t[:, ks, m * 128:(m + 1) * 128],
                                    cat[ks][:, fs], start=(ks == 0), stop=(ks == 3))
                            t = work.tile([128, 512], F32, tag="yh", name="yh")
                            nc.scalar.activation(t[:], ps[:], AF.Relu,
                                                 bias=projb_t[:, m, 0:1])
                            nc.vector.tensor_add(xT[m][:, fs], t[:], xT[m][:, fs])

                # stats pool closes before the rep pool opens: head PSUM peak
                # stays at 4 banks, leaving 4-7 free for the next iteration's
                # front phases (which the park pool pins there)
                mu_row = wpool.tile([1, TOK], F32, tag="mu_row", name="mu_row")
                rstd_row = wpool.tile([1, TOK], F32, tag="rstd_row", name="rstd_row")
                ones_col = wpool.tile([128, 1], F32, tag="ones_col2", name="ones_col2")
                nc.vector.memset(ones_col[:], 1.0)
                ones1 = wpool.tile([1, 128], F32, tag="ones1b", name="ones1b")
                nc.vector.memset(ones1[:], 1.0)
                with tc.tile_pool(name="pln1", bufs=1, space="PSUM") as pln1:
                    mu_ps = pln1.tile([1, TOK], F32, tag="mu", name="mu")
                    ss_ps = pln1.tile([1, TOK], F32, tag="ss2", name="ss2")
                    for fh in range(2):
                        fs = slice(fh * 512, (fh + 1) * 512)
                        for m in range(MT):
                            nc.tensor.matmul(mu_ps[:, fs], ones_col[:], xn2[m][:, fs],
                                             start=(m == 0), stop=(m == MT - 1))
                            sq = work.tile([128, 512], F32, tag="sqtmp", name="ln_sq")
                            nc.scalar.square(sq[:], xn2[m][:, fs])
                            nc.tensor.matmul(ss_ps[:, fs], ones_col[:], sq[:],
                                             start=(m == 0), stop=(m == MT - 1))
                    nc.scalar.mul(mu_row[:], mu_ps[:], 1.0 / DM)
                    # var = ss/DM - mu^2 (built in rstd_row, then rstd in place)
                    nc.scalar.mul(rstd_row[:], ss_ps[:], 1.0 / DM)
                    mu2 = work.tile([1, TOK], F32, tag="rowtmp", name="mu2")
                    nc.vector.tensor_mul(mu2[:], mu_row[:], mu_row[:])
                    nc.vector.tensor_sub(rstd_row[:], rstd_row[:], mu2[:])
                    eps2 = wpool.tile([1, 1], F32, tag="eps2", name="eps2")
                    nc.vector.memset(eps2[:], 1e-5)
                    nc.scalar.activation(rstd_row[:], rstd_row[:], AF.Ln, bias=eps2[:, 0:1])
                    nc.scalar.activation(rstd_row[:], rstd_row[:], AF.Exp, scale=-0.5)
                with tc.tile_pool(name="pln2", bufs=1, space="PSUM") as pln2:
                    mu_rep = pln2.tile([128, TOK], F32, tag="mu_rep", name="mu_rep")
                    rs_rep = pln2.tile([128, TOK], F32, tag="rs_rep2", name="rs_rep2")
                    for fh in range(2):
                        fs = slice(fh * 512, (fh + 1) * 512)
                        nc.tensor.matmul(mu_rep[:, fs], ones1[:], mu_row[:, fs],
                                         start=True, stop=True)
                        nc.tensor.matmul(rs_rep[:, fs], ones1[:], rstd_row[:, fs],
                                         start=True, stop=True)
                    for m in range(MT):
                        nc.vector.tensor_sub(xn2[m][:], xn2[m][:], mu_rep[:])
                        nc.vector.tensor_mul(xn2[m][:], xn2[m][:], rs_rep[:])
                        nc.scalar.activation(xn2[m][:], xn2[m][:], AF.Identity,
                                             bias=lnb_t[:, m, 0:1],
                                             scale=lng_t[:, m, 0:1])
                        nc.sync.dma_start(outT_d[m * 128:(m + 1) * 128, :], xn2[m][:])

            if loop_k > 1:
                with tc.For_i(0, loop_k, 1):
                    body()
            else:
                body()

    nc.compile()
    _BUILD_CACHE[key] = nc
    return nc


# ======================================================================
# host entry
# ======================================================================

def _make_in_maps(inputs):
    x = np.asarray(inputs["x"], F32_np)
    fw = _prep_layer_weights(inputs["fm_in"], inputs["fm_convw"], inputs["fm_convb"],
                             inputs["fm_xproj"], inputs["fm_dtw"], inputs["fm_dtb"],
                             inputs["fm_Alog"], inputs["fm_D"], inputs["fm_out"],
                             inputs["fm_norm"])
    bw = _prep_layer_weights(inputs["bm_in"], inputs["bm_convw"], inputs["bm_convb"],
                             inputs["bm_xproj"], inputs["bm_dtw"], inputs["bm_dtb"],
                             inputs["bm_Alog"], inputs["bm_D"], inputs["bm_out"],
                             inputs["bm_norm"])
    sh = _prep_shared_weights(inputs["proj_w"], inputs["proj_b"],
                              inputs["ln_g"], inputs["ln_b"])
    base = {}
    for s, w in (("f", fw), ("b", bw)):
        for k, v in w.items():
            base[f"{s}_{k}"] = v
    base["projw"] = sh["projw"]
    base["projb"] = sh["projb"]
    base["lng"] = sh["lng"]
    base["lnb"] = sh["lnb"]

    in_maps = []
    for c in range(NCORES):
        xc = x[c * BL:(c + 1) * BL]                       # (BL, L, DM)
        xTc = np.ascontiguousarray(xc.reshape(TOK, DM).T)  # (DM, TOK)
        m = dict(base)
        m["xT"] = xTc
        in_maps.append(m)
    return in_maps


def _unshard(results):
    outs = []
    for c in range(NCORES):
        oT = results[c]["outT"]                            # (DM, TOK)
        outs.append(np.ascontiguousarray(oT.T.reshape(BL, L, DM)))
    return np.concatenate(outs, axis=0).astype(F32_np)


def kernel(**inputs):
    from concourse import bass_utils
    nc = _build(loop_k=1)
    in_maps = _make_in_maps(inputs)
    res = bass_utils.run_bass_kernel_spmd(nc, in_maps, core_ids=list(range(NCORES)))
    return _unshard(res.results)
